# revision 6
# baseline (speedup 1.0000x reference)
"""Trainium2 Bass kernel for the decoupled-SISO block SSM.

Model (per reference):
  x_{t+1} = fx(x_t) + fu(u_t);  y_t = <Wfy, x_{t+1}> per channel
  fx: per-channel 3-layer MLP (8->8->8->8, gelu on hidden layers)
  fu: per-channel MLP on the scalar input (1->8->8->8, gelu on hidden)

Sharding (8 cores): 2-way over the 32 decoupled channels x 4-way over batch.
Each core owns 16 channels (128 state rows) x 128 batch and runs its 512-step
recurrence independently - zero cross-device traffic.

Per-core layout: state lives as [kh=128 partitions, batch=128 free].  The
per-channel 8x8 weight matrices are packed block-diagonally into [128,128]
stationaries so each step is dense 128-contraction matmuls.  The batch is
split into two 64-wide chains pipelined half a step apart so the two gelu
stages of fx from the two chains fuse into single [128,128] ACT instructions.
fu depends only on u, so it is computed a chunk (16 steps) ahead with
N=512 matmuls and large gelu tiles, then added into the state in PSUM->SBUF.
Outputs are staged in SBUF per 16-step chunk and DMA'd as ~1MB transfers.
"""

import os
import sys
from contextlib import ExitStack

import numpy as np

for _p in ("/opt/trn_rl_repo", "/root/.axon_site/_ro/trn_rl_repo"):
    if os.path.isdir(_p) and _p not in sys.path:
        sys.path.insert(0, _p)

import ml_dtypes  # noqa: E402

import concourse.bass as bass  # noqa: E402
import concourse.bacc as bacc  # noqa: E402
import concourse.tile as tile  # noqa: E402
from concourse import mybir  # noqa: E402
from concourse.bass_utils import run_bass_kernel_spmd  # noqa: E402

NSTEPS, B, NY, H = 512, 512, 32, 8
NSTEPS = int(os.environ.get("BASS_SSM_NSTEPS", str(NSTEPS)))  # dev knob
NX = NY * H
NCORE = 8
CH_SPLIT, B_SPLIT = 2, 4
CHP = NY // CH_SPLIT        # channels per core: 16
KH = CHP * H                # state rows per core: 128
BC = B // B_SPLIT           # batch per core: 128
HB = BC // 2                # batch per chain: 64
TC = 16                     # timesteps per output chunk
NCH = NSTEPS // TC          # chunks: 32
QT = 4                      # timesteps per fu quarter (N = QT*BC = 512)
NQ = TC // QT               # quarters per chunk: 4
FUW = QT * BC               # fu matmul moving width: 512

BF = mybir.dt.bfloat16
F32 = mybir.dt.float32
GELU = mybir.ActivationFunctionType.Gelu_apprx_tanh

_CACHE = {}


def _emit(ctx, tc, io):
    nc = tc.nc
    x0t, uft, wfx_d, wfu0_d, wfu1_d, wfu2_d, wfy_d, xo, fuo, yo = io

    wts = ctx.enter_context(tc.tile_pool(name="wts", bufs=1))
    stage = ctx.enter_context(tc.tile_pool(name="stage", bufs=2))
    fustage = ctx.enter_context(tc.tile_pool(name="fustage", bufs=3))
    uin_p = ctx.enter_context(tc.tile_pool(name="uin", bufs=3))
    zp_p = ctx.enter_context(tc.tile_pool(name="zp", bufs=4))
    xbf_p = ctx.enter_context(tc.tile_pool(name="xbf", bufs=6))
    fus_p = ctx.enter_context(tc.tile_pool(name="fus", bufs=2))
    psG = ctx.enter_context(tc.tile_pool(name="psG", bufs=2, space="PSUM"))
    psXn = ctx.enter_context(tc.tile_pool(name="psXn", bufs=2, space="PSUM"))
    psFu = ctx.enter_context(tc.tile_pool(name="psFu", bufs=2, space="PSUM"))
    psY = ctx.enter_context(tc.tile_pool(name="psY", bufs=2, space="PSUM"))

    # --- persistent weights -------------------------------------------------
    wfx = []
    for l in range(3):
        w = wts.tile([KH, KH], BF, tag=f"wfx{l}", name=f"wfx{l}")
        nc.sync.dma_start(out=w, in_=wfx_d[l])
        wfx.append(w)
    wfu0 = wts.tile([CHP, KH], BF, tag="wfu0")
    nc.sync.dma_start(out=wfu0, in_=wfu0_d[:])
    wfu1 = wts.tile([KH, KH], BF, tag="wfu1")
    nc.sync.dma_start(out=wfu1, in_=wfu1_d[:])
    wfu2 = wts.tile([KH, KH], BF, tag="wfu2")
    nc.sync.dma_start(out=wfu2, in_=wfu2_d[:])
    wfy = wts.tile([KH, CHP], BF, tag="wfy")
    nc.sync.dma_start(out=wfy, in_=wfy_d[:])

    # --- u input chunks (pre-transposed [t, k, b] in DRAM) ------------------
    uin = [None] * NCH

    def load_uin(c):
        t = uin_p.tile([CHP, TC, BC], BF, tag="uin", name=f"uin{c}")
        nc.sync.dma_start(out=t, in_=uft[c * TC:(c + 1) * TC].rearrange("t k b -> k t b"))
        uin[c] = t

    load_uin(0)
    if NCH > 1:
        load_uin(1)

    # --- initial state ------------------------------------------------------
    xbfA = xbf_p.tile([KH, HB], BF, tag="xbf")
    nc.sync.dma_start(out=xbfA, in_=x0t[:, 0:HB])
    xbfB = xbf_p.tile([KH, HB], BF, tag="xbf")
    nc.sync.dma_start(out=xbfB, in_=x0t[:, HB:BC])

    # --- chain-B prologue: z_B1(0) = gelu(W0 @ xB) --------------------------
    g0 = psG.tile([KH, BC], F32, tag="g")
    nc.tensor.matmul(g0[:, HB:BC], lhsT=wfx[0], rhs=xbfB, start=True, stop=True)
    zp0 = zp_p.tile([KH, BC], BF, tag="zp")
    nc.scalar.activation(zp0[:, HB:BC], g0[:, HB:BC], GELU)
    zB1 = zp0[:, HB:BC]

    # --- fu pipeline --------------------------------------------------------
    FUs = [None] * NCH

    def fu_head(c, q):
        """expand matmul + first gelu for quarter q of chunk c."""
        if FUs[c] is None:
            FUs[c] = fustage.tile([KH, TC, BC], F32, tag="fus", name=f"fus{c}")
        z0p = psFu.tile([KH, FUW], F32, tag="fup")
        nc.tensor.matmul(z0p, lhsT=wfu0, rhs=uin[c][:, q * QT:(q + 1) * QT, :],
                         start=True, stop=True)
        z0s = fus_p.tile([KH, FUW], BF, tag="z0s")
        nc.scalar.activation(z0s[:, 0:FUW // 2], z0p[:, 0:FUW // 2], GELU)
        return z0p, z0s

    def fu_g0b(st):
        z0p, z0s = st
        nc.scalar.activation(z0s[:, FUW // 2:FUW], z0p[:, FUW // 2:FUW], GELU)

    def fu_mid(st):
        z0p, z0s = st
        z1p = psFu.tile([KH, FUW], F32, tag="fup")
        nc.tensor.matmul(z1p, lhsT=wfu1, rhs=z0s, start=True, stop=True)
        z1s = fus_p.tile([KH, FUW], BF, tag="z1s")
        nc.scalar.activation(z1s[:, 0:FUW // 2], z1p[:, 0:FUW // 2], GELU)
        return z1p, z1s

    def fu_g1b(st):
        z1p, z1s = st
        nc.scalar.activation(z1s[:, FUW // 2:FUW], z1p[:, FUW // 2:FUW], GELU)

    def fu_tail(c, q, st):
        z1p, z1s = st
        fup = psFu.tile([KH, FUW], F32, tag="fup")
        nc.tensor.matmul(fup, lhsT=wfu2, rhs=z1s, start=True, stop=True)
        nc.vector.tensor_copy(
            out=FUs[c][:, q * QT:(q + 1) * QT, :].rearrange("p t b -> p (t b)"),
            in_=fup)

    def fu_quarter_serial(c, q):
        st = fu_head(c, q)
        fu_g0b(st)
        st2 = fu_mid(st)
        fu_g1b(st2)
        fu_tail(c, q, st2)

    # chunk 0's fu entirely in the prologue
    for q in range(NQ):
        fu_quarter_serial(0, q)

    # --- output staging -----------------------------------------------------
    Xs = [None] * NCH
    Ys = [None] * NCH

    def dma_out(c):
        nc.sync.dma_start(out=xo[c * TC:(c + 1) * TC].rearrange("t p b -> p t b"),
                          in_=Xs[c])
        nc.sync.dma_start(out=fuo[c * TC:(c + 1) * TC].rearrange("t p b -> p t b"),
                          in_=FUs[c])
        nc.sync.dma_start(out=yo[c * TC:(c + 1) * TC].rearrange("t k b -> k t b"),
                          in_=Ys[c])

    # --- main recurrence ----------------------------------------------------
    pend_tail = None        # (c, q, st) for fu_tail at next insert point
    pend_mid = None         # (c, q, st) state between head and mid
    ypsum = None

    for t in range(NSTEPS):
        c, t_loc = divmod(t, TC)
        q, p = divmod(t_loc, QT)
        last = t == NSTEPS - 1

        if t_loc == 0:
            if c > 0:
                dma_out(c - 1)
            if c + 2 < NCH:
                load_uin(c + 2)
            Xs[c] = stage.tile([KH, TC, BC], F32, tag="xs", name=f"xs{c}")
            Ys[c] = stage.tile([CHP, TC, BC], F32, tag="ys", name=f"ys{c}")
        if t_loc % QT == 0:
            ypsum = psY.tile([CHP, QT, BC], F32, tag="y")

        fc = c + 1  # fu chunk being computed during this chunk

        # fx first half-step: mm1_A(t), mm2_B(t) -> fused gelu
        g1 = psG.tile([KH, BC], F32, tag="g")
        nc.tensor.matmul(g1[:, 0:HB], lhsT=wfx[0], rhs=xbfA, start=True, stop=True)
        nc.tensor.matmul(g1[:, HB:BC], lhsT=wfx[1], rhs=zB1, start=True, stop=True)
        zp1 = zp_p.tile([KH, BC], BF, tag="zp")
        nc.scalar.activation(zp1, g1, GELU)

        # interleave fu work for chunk c+1 (gap-fills PE/ACT behind fuse1)
        if fc < NCH:
            if p == 0:
                if pend_tail is not None:
                    fu_tail(*pend_tail)
                    pend_tail = None
                pend_mid = (fc, q, fu_head(fc, q))
            elif p == 1:
                fu_g0b(pend_mid[2])
            elif p == 2:
                st2 = fu_mid(pend_mid[2])
                pend_mid = (pend_mid[0], pend_mid[1], None)
                pend_tail = (fc, q, st2)
            elif p == 3:
                fu_g1b(pend_tail[2])
        elif pend_tail is not None:
            fu_tail(*pend_tail)
            pend_tail = None

        # second half-step: mm2_A(t), mm3_B(t); fused gelu over [zA2 | zB1(t+1)]
        g2 = psG.tile([KH, BC], F32, tag="g")
        nc.tensor.matmul(g2[:, 0:HB], lhsT=wfx[1], rhs=zp1[:, 0:HB], start=True, stop=True)
        xnB = psXn.tile([KH, HB], F32, tag="xn")
        nc.tensor.matmul(xnB, lhsT=wfx[2], rhs=zp1[:, HB:BC], start=True, stop=True)
        nc.vector.tensor_add(Xs[c][:, t_loc, HB:BC], xnB, FUs[c][:, t_loc, HB:BC])
        xbfB = xbf_p.tile([KH, HB], BF, tag="xbf")
        nc.gpsimd.tensor_copy(out=xbfB, in_=Xs[c][:, t_loc, HB:BC])
        if not last:
            nc.tensor.matmul(g2[:, HB:BC], lhsT=wfx[0], rhs=xbfB, start=True, stop=True)
        nc.tensor.matmul(ypsum[:, p, HB:BC], lhsT=wfy, rhs=xbfB, start=True, stop=True)

        zp2 = zp_p.tile([KH, BC], BF, tag="zp")
        if not last:
            nc.scalar.activation(zp2, g2, GELU)
        else:
            nc.scalar.activation(zp2[:, 0:HB], g2[:, 0:HB], GELU)

        # close step: mm3_A(t)
        xnA = psXn.tile([KH, HB], F32, tag="xn")
        nc.tensor.matmul(xnA, lhsT=wfx[2], rhs=zp2[:, 0:HB], start=True, stop=True)
        nc.vector.tensor_add(Xs[c][:, t_loc, 0:HB], xnA, FUs[c][:, t_loc, 0:HB])
        xbfA = xbf_p.tile([KH, HB], BF, tag="xbf")
        nc.gpsimd.tensor_copy(out=xbfA, in_=Xs[c][:, t_loc, 0:HB])
        nc.tensor.matmul(ypsum[:, p, 0:HB], lhsT=wfy, rhs=xbfA, start=True, stop=True)
        zB1 = zp2[:, HB:BC]

        if p == QT - 1:
            nc.vector.tensor_copy(
                out=Ys[c][:, q * QT:(q + 1) * QT, :].rearrange("k t b -> k (t b)"),
                in_=ypsum.rearrange("k t b -> k (t b)"))

    dma_out(NCH - 1)


def _build():
    nc = bacc.Bacc("TRN2", target_bir_lowering=False, debug=False,
                   enable_asserts=False)
    x0t = nc.declare_dram_parameter("x0t", [KH, BC], BF, isOutput=False).ap()
    uft = nc.declare_dram_parameter("uft", [NSTEPS, CHP, BC], BF, isOutput=False).ap()
    wfx = nc.declare_dram_parameter("wfx", [3, KH, KH], BF, isOutput=False).ap()
    wfu0 = nc.declare_dram_parameter("wfu0", [CHP, KH], BF, isOutput=False).ap()
    wfu1 = nc.declare_dram_parameter("wfu1", [KH, KH], BF, isOutput=False).ap()
    wfu2 = nc.declare_dram_parameter("wfu2", [KH, KH], BF, isOutput=False).ap()
    wfy = nc.declare_dram_parameter("wfy", [KH, CHP], BF, isOutput=False).ap()
    xo = nc.declare_dram_parameter("xo", [NSTEPS, KH, BC], F32, isOutput=True).ap()
    fuo = nc.declare_dram_parameter("fuo", [NSTEPS, KH, BC], F32, isOutput=True).ap()
    yo = nc.declare_dram_parameter("yo", [NSTEPS, CHP, BC], F32, isOutput=True).ap()
    io = (x0t, uft, wfx, wfu0, wfu1, wfu2, wfy, xo, fuo, yo)

    with tile.TileContext(nc) as tc:
        with ExitStack() as ctx:
            _emit(ctx, tc, io)
    nc.compile()
    return nc


def _get_program():
    if "nc" not in _CACHE:
        _CACHE["nc"] = _build()
    return _CACHE["nc"]


def _bf(a):
    return np.ascontiguousarray(a).astype(ml_dtypes.bfloat16)


def _make_in_maps(x0, Uf, Wfx, Wfu0, Wfu1, Wfu2, Wfy):
    # per-channel-group preprocessed weights
    wmaps = []
    for cg in range(CH_SPLIT):
        ks = slice(cg * CHP, (cg + 1) * CHP)
        wfx_bd = np.zeros((3, KH, KH), np.float32)
        wfu1_bd = np.zeros((KH, KH), np.float32)
        wfu2_bd = np.zeros((KH, KH), np.float32)
        wfu0_e = np.zeros((CHP, KH), np.float32)
        wfy_e = np.zeros((KH, CHP), np.float32)
        for k in range(CHP):
            sl = slice(k * H, (k + 1) * H)
            for l in range(3):
                wfx_bd[l, sl, sl] = Wfx[cg * CHP + k, l]
            wfu1_bd[sl, sl] = Wfu1[cg * CHP + k]
            wfu2_bd[sl, sl] = Wfu2[cg * CHP + k]
            wfu0_e[k, sl] = Wfu0[cg * CHP + k]
            wfy_e[sl, k] = Wfy[cg * CHP + k]
        wmaps.append({
            "wfx": _bf(wfx_bd), "wfu0": _bf(wfu0_e), "wfu1": _bf(wfu1_bd),
            "wfu2": _bf(wfu2_bd), "wfy": _bf(wfy_e),
        })

    in_maps = []
    Uf = Uf[:NSTEPS]
    for cid in range(NCORE):
        cg, bg = divmod(cid, B_SPLIT)
        bs = slice(bg * BC, (bg + 1) * BC)
        x0t = _bf(x0[bs, cg * KH:(cg + 1) * KH].T)                 # [KH, BC]
        uft = _bf(Uf[:, bs, cg * CHP:(cg + 1) * CHP].transpose(0, 2, 1))  # [t,k,b]
        in_maps.append({"x0t": x0t, "uft": uft, **wmaps[cg]})
    return in_maps


def _assemble(results):
    X = np.empty((NSTEPS, B, NX), np.float32)
    FU = np.empty((NSTEPS, B, NX), np.float32)
    Y = np.empty((NSTEPS, B, NY), np.float32)
    for cid in range(NCORE):
        cg, bg = divmod(cid, B_SPLIT)
        bs = slice(bg * BC, (bg + 1) * BC)
        r = results[cid]
        X[:, bs, cg * KH:(cg + 1) * KH] = r["xo"].transpose(0, 2, 1)
        FU[:, bs, cg * KH:(cg + 1) * KH] = r["fuo"].transpose(0, 2, 1)
        Y[:, bs, cg * CHP:(cg + 1) * CHP] = r["yo"].transpose(0, 2, 1)
    return X, Y, FU


def run(inputs, trace=False, **kw):
    nc = _get_program()
    in_maps = _make_in_maps(inputs["x0"], inputs["Uf"], inputs["Wfx"],
                            inputs["Wfu0"], inputs["Wfu1"], inputs["Wfu2"],
                            inputs["Wfy"])
    res = run_bass_kernel_spmd(nc, in_maps, core_ids=list(range(NCORE)),
                               trace=trace, **kw)
    return _assemble(res.results), res


def kernel(**inputs):
    (X, Y, FU), _ = run(inputs, trace=False)
    return X, Y, FU
